# revision 4
# baseline (speedup 1.0000x reference)
"""LookAheadMask kernel for Trainium2.

out[b, r, c] = 1.0 if c > r else x[b, r, c], for x of shape (8, 4096, 4096) f32.

Sharding: batch dim across 8 NeuronCores (data parallel, no communication).

Per-core plan (matrix is S x S, S=4096, row-blocks of P=128), raw bass:

  - strictly-lower region (cols < block start): 31 direct DRAM->DRAM copies
  - strictly-upper region (cols >= block end): 31 DMAs from an SBUF ones tile
  - the 32 diagonal 128x128 blocks: one 3D-strided gather DMA (exact 128-col
    windows) into SBUF [128, 32*128], one gpsimd affine_select (keep x where
    row >= col-within-block, else 1.0), one scatter back.

Profiling facts this schedule is built on (ntff trace of the previous
version):
  - The 16 SDMA engines are the shared wall: ~24.5 B/ns per engine on this
    packet mix (~392 GB/s aggregate over both HWDGE rings).
  - HWDGE descriptor generation is ~10.5 ns/desc, serial per ring. The
    gather/scatter are 4096 x 512B descriptors each (~43 us of generation),
    so they must overlap the other ring's bulk data, never sit at the end.
  - Engines round-robin per packet across rings, so a ring generating small
    descriptors starves unless the other ring has big packets queued.

Schedule:
  SP   : copies i=31..16 (big first) | wait msem | ones i=0..7 | wait asem |
         scatter (gen overlaps ACT's remaining ones stream) | drain
  ACT  : gather (gen overlaps SP's copy stream) | copies i=15..1 |
         wait msem | ones i=8..30 | drain
  GPSIMD: wait gather, affine_select diag blocks
  VECTOR: memset ones tile

HBM traffic/core: ~34.6 MiB read + 64 MiB write; ~66 MiB through the SDMA
engines (vs 68 before), with no descriptor-generation tail.
"""

import numpy as np

from concourse import bass, mybir
from concourse.bass_utils import run_bass_kernel_spmd

S = 4096
P = 128
NB = S // P  # 32
N_CORES = 8

# DMAs incrementing dsem: 31 copies + 31 ones + scatter (gather incs gsem).
TOTAL_DMA_INC = 16 * 63

_cached_nc = None


def _build():
    global _cached_nc
    if _cached_nc is not None:
        return _cached_nc

    nc = bass.Bass()
    x = nc.dram_tensor("x", [S, S], mybir.dt.float32, kind="ExternalInput")
    out = nc.dram_tensor("out", [S, S], mybir.dt.float32, kind="ExternalOutput")

    # Diagonal-block view: [row-in-block(128), block(32), col-in-block(128)],
    # block b starts at element offset b*(P*S + P). Strides in elements.
    diag_pairs = [[S, P], [P * S + P, NB], [1, P]]

    def copy_blk(eng, i, dsem):
        r0 = i * P
        eng.dma_start(
            out=out[r0 : r0 + P, 0:r0], in_=x[r0 : r0 + P, 0:r0]
        ).then_inc(dsem, 16)

    def ones_blk(eng, i, ones, dsem):
        r0 = i * P
        w = S - r0 - P
        eng.dma_start(
            out=out[r0 : r0 + P, r0 + P : S], in_=ones[:, :w]
        ).then_inc(dsem, 16)

    with (
        nc.Block() as block,
        nc.semaphore("dsem") as dsem,  # all DMA completions
        nc.semaphore("gsem") as gsem,  # diag gather done
        nc.semaphore("msem") as msem,  # ones memset done
        nc.semaphore("asem") as asem,  # affine_select done
        nc.sbuf_tensor("ones", [P, S], mybir.dt.float32) as ones,
        nc.sbuf_tensor("diag_in", [P, S], mybir.dt.float32) as diag_in,
        nc.sbuf_tensor("diag_out", [P, S], mybir.dt.float32) as diag_out,
    ):

        @block.vector
        def _(vector: bass.BassVectorEngine):
            vector.memset(ones[:, :], 1.0).then_inc(msem, 1)

        @block.sync
        def _(sync: bass.BassEngine):
            for i in range(NB - 1, 15, -1):  # 31..16, biggest copies first
                copy_blk(sync, i, dsem)
            sync.wait_ge(msem, 1)
            for i in range(0, 8):  # biggest ones first
                ones_blk(sync, i, ones, dsem)
            sync.wait_ge(asem, 1)
            sync.dma_start(
                out=bass.AP(out, 0, diag_pairs), in_=diag_out[:, :]
            ).then_inc(dsem, 16)
            sync.wait_ge(dsem, TOTAL_DMA_INC)

        @block.scalar
        def _(scalar: bass.BassEngine):
            scalar.dma_start(
                out=diag_in[:, :], in_=bass.AP(x, 0, diag_pairs)
            ).then_inc(gsem, 16)
            for i in range(15, 0, -1):
                copy_blk(scalar, i, dsem)
            scalar.wait_ge(msem, 1)
            for i in range(8, NB - 1):
                ones_blk(scalar, i, ones, dsem)
            scalar.wait_ge(dsem, TOTAL_DMA_INC)

        @block.gpsimd
        def _(gpsimd: bass.BassGpSimd):
            gpsimd.wait_ge(gsem, 16)
            # iota[p, (b, c)] = p - c; keep x where >= 0 (at/below diag).
            gpsimd.affine_select(
                out=diag_out[:, :],
                in_=bass.AP(diag_in, 0, [[S, P], [P, NB], [1, P]]),
                pattern=[[0, NB], [-1, P]],
                base=0,
                channel_multiplier=1,
                compare_op=mybir.AluOpType.is_ge,
                fill=1.0,
            ).then_inc(asem, 1)

    _cached_nc = nc
    return nc


def _run(x_full: np.ndarray, trace: bool = False):
    nc = _build()
    x_full = np.asarray(x_full, dtype=np.float32)
    in_maps = [{"x": x_full[i]} for i in range(N_CORES)]
    res = run_bass_kernel_spmd(nc, in_maps, list(range(N_CORES)), trace=trace)
    out = np.stack([res.results[i]["out"] for i in range(N_CORES)], axis=0)
    return out, res


def kernel(x: np.ndarray) -> np.ndarray:
    out, _ = _run(x, trace=False)
    return out


# revision 6
# speedup vs baseline: 1.2445x; 1.2445x over previous
"""LookAheadMask kernel for Trainium2.

out[b, r, c] = 1.0 if c > r else x[b, r, c], for x of shape (8, 4096, 4096) f32.

Sharding: batch dim across 8 NeuronCores (data parallel, no communication).

Per-core plan (matrix is S x S, S=4096, row-blocks of P=128), raw bass.

Trace facts this schedule is built on (ntff profiles of prior versions):
  - 16 SDMA engines serve both HWDGE rings, round-robin per packet; a
    queue's byte share is proportional to its packet size vs the other
    queue's, so small-packet streams starve next to big ones.
  - SBUF->DRAM big packets run ~26.5 B/ns/engine; D2D copies ~20 B/ns.
  - A trailing 4096x512B scatter is descriptor-generation limited
    (~10.5 ns/desc = 43 us) with idle engines - this version has none.
  - Engine 15 runs ~19% slower on SBUF-sourced streams; any barrier
    exposes it as a stall, so the schedule has NO mid-kernel barriers.

Structure:
  - strictly-lower region: 31 D2D copies (ascending size) on the SP ring,
    overlapped with the diag gather (1 KB packets) on the ACT ring.
  - diagonal 128x128 blocks ride the upper writes: UB[128, 32*512] holds
    32 slots of 512 cols, all ones except cols [0:128) of slot i = the
    affine-selected diag block i (one gpsimd affine_select, 3D APs).
    Merged write i covers out[r0:r0+P, r0:r0+512] from slot i (2 KB
    descriptors); plain ones DMAs cover out[r0:r0+P, r0+512:S] from a
    ones tile, ungated by the gather/affine.

Engine programs:
  SP    : 31 copies asc | wait affine | 16 even merged writes | drain
  ACT   : gather | wait memset | 28 plain ones asc | wait affine |
          16 odd merged writes | drain
  GPSIMD: wait gather+memset; one affine_select for all 32 slots
  VECTOR: memset UB then ones tile (~17 us, hidden under copies)

HBM traffic/core: ~35 MiB read + 64 MiB write, ~71 MB through the SDMA
engines, no descriptor-generation tail, one straggler exposure at the end.
"""

import numpy as np

from concourse import bass, mybir
from concourse.bass_utils import run_bass_kernel_spmd

S = 4096
P = 128
NB = S // P  # 32
N_CORES = 8
W = 256  # gather window cols per diag block (1 KB descriptors)
WU = 512  # merged diag+ones write width (2 KB descriptors)
PO = S - WU  # 3584: plain-ones tile width

DSEM_TARGET = 16 * (31 + 28)  # copies + plain-ones
USEM_TARGET = 16 * NB  # merged writes

_cached_nc = None


def _build():
    global _cached_nc
    if _cached_nc is not None:
        return _cached_nc

    nc = bass.Bass()
    x = nc.dram_tensor("x", [S, S], mybir.dt.float32, kind="ExternalInput")
    out = nc.dram_tensor("out", [S, S], mybir.dt.float32, kind="ExternalOutput")

    with (
        nc.Block() as block,
        nc.semaphore("dsem") as dsem,  # copy + plain-ones DMA completions
        nc.semaphore("gsem") as gsem,  # diag gather done
        nc.semaphore("msem") as msem,  # memsets done
        nc.semaphore("asem") as asem,  # affine_select done
        nc.semaphore("usem") as usem,  # merged-write DMA completions
        nc.sbuf_tensor("ub", [P, NB * WU], mybir.dt.float32) as ub,
        nc.sbuf_tensor("ones", [P, PO], mybir.dt.float32) as ones,
        nc.sbuf_tensor("diag_in", [P, NB * W], mybir.dt.float32) as diag_in,
    ):

        def merged_write(eng, i):
            """Block-row i's diag block + first ones cols in one DMA."""
            r0 = i * P
            w = min(WU, S - r0)
            eng.dma_start(
                out=out[r0 : r0 + P, r0 : r0 + w],
                in_=ub[:, i * WU : i * WU + w],
            ).then_inc(usem, 16)

        @block.vector
        def _(vector: bass.BassVectorEngine):
            vector.memset(ub[:, :], 1.0)
            vector.memset(ones[:, :], 1.0).then_inc(msem, 1)

        @block.sync
        def _(sync: bass.BassEngine):
            for i in range(1, NB):  # ascending size D2D copies
                r0 = i * P
                sync.dma_start(
                    out=out[r0 : r0 + P, 0:r0], in_=x[r0 : r0 + P, 0:r0]
                ).then_inc(dsem, 16)
            sync.wait_ge(asem, 1)
            for i in range(0, NB, 2):
                merged_write(sync, i)
            sync.wait_ge(dsem, DSEM_TARGET)
            sync.wait_ge(usem, USEM_TARGET)

        @block.scalar
        def _(scalar: bass.BassEngine):
            # Gather window: W cols per diag block ending at its right edge
            # (1 KB descriptors). Blocks 1..31 in one DMA; block 0's window
            # would start before the tensor, so it gets its own 128-col load.
            scalar.dma_start(
                out=bass.AP(diag_in, W, [[NB * W, P], [W, NB - 1], [1, W]]),
                in_=bass.AP(
                    x, (P * S + P) + P - W, [[S, P], [P * S + P, NB - 1], [1, W]]
                ),
            ).then_inc(gsem, 16)
            scalar.dma_start(
                out=bass.AP(diag_in, W - P, [[NB * W, P], [1, P]]),
                in_=x[0:P, 0:P],
            ).then_inc(gsem, 16)
            scalar.wait_ge(msem, 1)
            for i in range(0, NB - 4):  # blocks 0..27 have cols past r0+WU
                r0 = i * P
                scalar.dma_start(
                    out=out[r0 : r0 + P, r0 + WU : S],
                    in_=ones[:, : S - r0 - WU],
                ).then_inc(dsem, 16)
            scalar.wait_ge(asem, 1)
            for i in range(1, NB, 2):
                merged_write(scalar, i)
            scalar.wait_ge(dsem, DSEM_TARGET)
            scalar.wait_ge(usem, USEM_TARGET)

        @block.gpsimd
        def _(gpsimd: bass.BassGpSimd):
            gpsimd.wait_ge(gsem, 32)
            gpsimd.wait_ge(msem, 1)
            # iota[p, (b, c)] = p - c; keep x where >= 0 (at/below diag),
            # else 1.0. Input reads the last 128 cols of each W-wide gathered
            # window; output lands at the head of UB slot b.
            gpsimd.affine_select(
                out=bass.AP(ub, 0, [[NB * WU, P], [WU, NB], [1, P]]),
                in_=bass.AP(diag_in, W - P, [[NB * W, P], [W, NB], [1, P]]),
                pattern=[[0, NB], [-1, P]],
                base=0,
                channel_multiplier=1,
                compare_op=mybir.AluOpType.is_ge,
                fill=1.0,
            ).then_inc(asem, 1)

    _cached_nc = nc
    return nc


def _run(x_full: np.ndarray, trace: bool = False):
    nc = _build()
    x_full = np.asarray(x_full, dtype=np.float32)
    in_maps = [{"x": x_full[i]} for i in range(N_CORES)]
    res = run_bass_kernel_spmd(nc, in_maps, list(range(N_CORES)), trace=trace)
    out = np.stack([res.results[i]["out"] for i in range(N_CORES)], axis=0)
    return out, res


def kernel(x: np.ndarray) -> np.ndarray:
    out, _ = _run(x, trace=False)
    return out
